# revision 1
# baseline (speedup 1.0000x reference)
"""MultiHeadAttention (B=2,N=2048,C=1024,H=16,Dk=64) on 8 TRN2 cores.

Head-tensor-parallel: core c owns heads {2c,2c+1} for both batches.
Device computes qkv^T = Wqkv_s^T @ x^T, causal softmax(q k^T/8) @ v, and the
partial out-projection (rows 128c:128c+128 of W_out); host sums the 8
partials (the "all-reduce"), transposes, and adds the fused bias.
b_k drops (softmax shift invariance); b_v folds into the output bias.
"""
import sys

sys.path.insert(0, "/opt/trn_rl_repo")
import numpy as np
import ml_dtypes
import concourse.bass as bass
import concourse.mybir as mybir
from concourse.bass_utils import run_bass_kernel_spmd
from concourse.tile import TileContext

F32 = mybir.dt.float32
F16 = mybir.dt.float16
BF16 = mybir.dt.bfloat16
AF = mybir.ActivationFunctionType
BF = ml_dtypes.bfloat16

T = 4096  # total tokens (2 batches x 2048)
TRACE = False
LAST_EXEC_NS = None
LAST_MEAN_NS = None

_MAX_WAITS = 1  # this neuronxcc build rejects instructions with more sem waits


def _split_excess_waits(nc, limit=_MAX_WAITS):
    """Move excess sem waits onto same-engine nops inserted just before the
    over-subscribed instruction (waits-before-inst on the same queue is
    semantically identical)."""
    ifaces = [nc.tensor, nc.scalar, nc.vector, nc.gpsimd, nc.sync]
    eng_map = {iface.engine: iface for iface in ifaces}
    f = nc.m.functions[0]
    for bb in list(f.blocks):
        il = bb.instructions
        i = 0
        while i < len(il):
            ins = il[i]
            si = ins.sync_info
            waits = list(si.on_wait) if si is not None else []
            if len(waits) > limit:
                keep = waits[-limit:]
                rest = waits[:-limit]
                ins.sync_info = mybir.SyncInfo(
                    on_wait=keep, on_update=list(si.on_update)
                )
                nops = []
                for k in range(0, len(rest), limit):
                    nop = eng_map[ins.engine].nop(nofuse=True)
                    nop.ins.sync_info = mybir.SyncInfo(
                        on_wait=rest[k : k + limit], on_update=[]
                    )
                    nops.append(nop.ins)
                for ni in nops:
                    for bb2 in list(f.blocks):
                        try:
                            bb2.instructions.remove(ni)
                            break
                        except ValueError:
                            pass
                for off, ni in enumerate(nops):
                    il.insert(i + off, ni)
                i += len(nops)
            i += 1


def _build():
    nc = bass.Bass("TRN2", target_bir_lowering=False, debug=False, num_devices=8)
    xt_d = nc.declare_dram_parameter("xt", (1024, T), BF16, isOutput=False)
    wqkv_d = nc.declare_dram_parameter("wqkv", (1024, 384), BF16, isOutput=False)
    bq_d = nc.declare_dram_parameter("bq", (128, 1), F32, isOutput=False)
    wout_d = nc.declare_dram_parameter("wout", (128, 1024), BF16, isOutput=False)
    tri_d = nc.declare_dram_parameter("tri", (128, 128), BF16, isOutput=False)
    sel_d = nc.declare_dram_parameter("sel", (16, 1024), BF16, isOutput=False)
    ident_d = nc.declare_dram_parameter("ident", (128, 128), BF16, isOutput=False)
    outp_d = nc.declare_dram_parameter("outp", (1024, T), F16, isOutput=True)

    with TileContext(nc) as tc:
        with tc.tile_pool(name="sb", bufs=1) as sb:
            # ---- constant / persistent tiles + input DMAs ----
            # two queues, inputs fully landed before compute: overlapping the
            # xt stream with phase A measured ~20% slower on every engine
            wq_t = [
                sb.tile((128, 384), BF16, tag=f"wq{kc}", name=f"wq{kc}")
                for kc in range(8)
            ]
            xt_t = [
                sb.tile((128, T), BF16, tag=f"xt{kc}", name=f"xt{kc}")
                for kc in range(8)
            ]
            bq_t = sb.tile((128, 1), F32, tag="bq")
            wout_t = sb.tile((128, 1024), BF16, tag="wout")
            tri_t = sb.tile((128, 128), BF16, tag="tri")
            id_t = sb.tile((128, 128), BF16, tag="ident")
            S_all = sb.tile((16, 1024), BF16, tag="sel")

            for kc in range(8):
                nc.sync.dma_start(wq_t[kc][:], wqkv_d[128 * kc : 128 * kc + 128, :])
            nc.gpsimd.dma_start(bq_t[:], bq_d[:, :])
            nc.gpsimd.dma_start(wout_t[:], wout_d[:, :])
            nc.gpsimd.dma_start(tri_t[:], tri_d[:, :])
            nc.gpsimd.dma_start(id_t[:], ident_d[:, :])
            for kc in range(8):
                eng = nc.sync if kc % 2 == 0 else nc.gpsimd
                eng.dma_start(xt_t[kc][:], xt_d[128 * kc : 128 * kc + 128, :])
            nc.gpsimd.dma_start(S_all[:], sel_d[:, :])

            q_T = sb.tile((128, T), BF16, tag="q_T")
            k_T = sb.tile((128, T), BF16, tag="k_T")
            v_T = sb.tile((128, T), BF16, tag="v_T")
            vext = [
                sb.tile((128, 2080), BF16, tag=f"vext{b}", name=f"vext{b}")
                for b in range(2)
            ]
            nc.vector.memset(vext[0][:], 1.0)
            nc.vector.memset(vext[1][:], 1.0)

            # es buffers for diagonal key-blocks: fully-masked columns are
            # zeroed once and never rewritten (exp writes only unmasked cols)
            es_diag = [
                sb.tile((128, 1024), BF16, tag=f"esd{r}", name=f"esd{r}")
                for r in range(4)
            ]
            for r in range(1, 4):
                nc.gpsimd.memset(es_diag[r][:, 0 : 128 * r], 0.0)
                nc.gpsimd.memset(es_diag[r][:, 512 : 512 + 128 * r], 0.0)

            # unnormalized attention outputs (row 64 = softmax denominator)
            av_sb = [
                sb.tile((65, 1024), F32, tag=f"avsb{t}", name=f"avsb{t}")
                for t in range(8)
            ]
            den16 = sb.tile((16, 512), F32, tag="den16")
            rec16 = sb.tile((16, 512), F32, tag="rec16")

            # ---- phase A: qkv^T = wqkv_s^T @ x^T ----
            with tc.tile_pool(name="psA", bufs=1, space="PSUM") as psA:
                dst = [q_T, k_T, v_T]
                for m in range(3):
                    chs = [
                        psA.tile((128, 512), F32, tag=f"ch{n}", name=f"ch{n}")
                        for n in range(8)
                    ]
                    for kc in range(8):
                        for n in range(8):
                            nc.tensor.matmul(
                                chs[n][:],
                                wq_t[kc][:, 128 * m : 128 * m + 128],
                                xt_t[kc][:, 512 * n : 512 * n + 512],
                                start=(kc == 0),
                                stop=(kc == 7),
                            )
                    for n in range(8):
                        o = dst[m][:, 512 * n : 512 * n + 512]
                        if m == 0:
                            nc.vector.tensor_scalar_add(o, chs[n][:], bq_t[:, 0:1])
                        else:
                            nc.scalar.activation(o, chs[n][:], AF.Copy)

            # ---- phase V: transpose v_T into [token, dim] blocks with a
            # trailing ones column per 65-wide block (softmax denominator) ----
            with tc.tile_pool(name="psV", bufs=1, space="PSUM") as psV:
                for t in range(32):
                    b, jj = divmod(t, 16)
                    trp = psV.tile((128, 128), BF16, tag="tr", bufs=2)
                    nc.tensor.transpose(trp[:], v_T[:, 128 * t : 128 * t + 128], id_t[:])
                    # single DVE copy lands both halves: out chunks at 65*jj
                    # and 65*(16+jj) (stride 1040), keeping ACT free for exps
                    c0 = 65 * jj
                    oslc = vext[b][:, c0 : c0 + 64]
                    islc = trp[:]
                    o_ap = bass.AP(
                        oslc.tensor,
                        oslc.offset,
                        [[oslc.ap[0][0], oslc.ap[0][1]], [1040, 2], [1, 64]],
                    )
                    i_ap = bass.AP(
                        islc.tensor,
                        islc.offset,
                        [[islc.ap[0][0], islc.ap[0][1]], [64, 2], [1, 64]],
                    )
                    nc.vector.tensor_copy(o_ap, i_ap)

            # ---- phase B: causal attention (unnormalized) ----
            with tc.tile_pool(name="psB", bufs=1, space="PSUM") as psB:
                for t in range(8):
                    b, i = divmod(t, 4)
                    av = [
                        psB.tile(
                            (65, 512), F32, tag=f"av{hl}", name=f"av{hl}", bufs=2
                        )
                        for hl in range(2)
                    ]
                    nj = 4 * i + 4
                    qs = 2048 * b + 512 * i
                    for jj in range(nj):
                        sps = psB.tile((128, 1024), F32, tag="sps", bufs=2)
                        ks = 2048 * b + 128 * jj
                        for hl in range(2):
                            nc.tensor.matmul(
                                sps[:, 512 * hl : 512 * hl + 512],
                                k_T[64 * hl : 64 * hl + 64, ks : ks + 128],
                                q_T[64 * hl : 64 * hl + 64, qs : qs + 512],
                                start=True,
                                stop=True,
                                skip_group_check=True,
                            )
                        r = jj - 4 * i
                        if r < 0:
                            es = sb.tile((128, 1024), BF16, tag="es", bufs=3)
                            nc.scalar.activation(es[:], sps[:], AF.Exp, scale=0.125)
                        else:
                            es = es_diag[r]
                            if r == 0:
                                nc.scalar.activation(
                                    es[:], sps[:], AF.Exp, scale=0.125
                                )
                            else:
                                # one 2-chunk ACT call covers both hl halves
                                w = 512 - 128 * r
                                oslc = es[:, 128 * r : 128 * r + w]
                                islc = sps[:, 128 * r : 128 * r + w]
                                o_ap = bass.AP(
                                    oslc.tensor,
                                    oslc.offset,
                                    [
                                        [oslc.ap[0][0], oslc.ap[0][1]],
                                        [512, 2],
                                        [1, w],
                                    ],
                                )
                                i_ap = bass.AP(
                                    islc.tensor,
                                    islc.offset,
                                    [
                                        [islc.ap[0][0], islc.ap[0][1]],
                                        [512, 2],
                                        [1, w],
                                    ],
                                )
                                nc.scalar.activation(
                                    o_ap, i_ap, AF.Exp, scale=0.125
                                )
                            for hl in range(2):
                                c0 = 512 * hl + 128 * r
                                nc.vector.tensor_mul(
                                    es[:, c0 : c0 + 128],
                                    es[:, c0 : c0 + 128],
                                    tri_t[:],
                                )
                        for hl in range(2):
                            c = 65 * (16 * hl + jj)
                            nc.tensor.matmul(
                                av[hl][:],
                                vext[b][:, c : c + 65],
                                es[:, 512 * hl : 512 * hl + 512],
                                start=(jj == 0),
                                stop=(jj == nj - 1),
                                skip_group_check=True,
                            )
                    # evacuate + collect denominators (row 64) -- SBUF->SBUF DMA
                    # because compute engines need quadrant-aligned partitions
                    for hl in range(2):
                        nc.vector.tensor_copy(
                            av_sb[t][:, 512 * hl : 512 * hl + 512], av[hl][:]
                        )
                        eng = nc.sync if hl == 0 else nc.gpsimd
                        eng.dma_start(
                            den16[2 * t + hl : 2 * t + hl + 1, :],
                            av_sb[t][64:65, 512 * hl : 512 * hl + 512],
                        )
                nc.vector.reciprocal(rec16[:], den16[:])
                # Dekker hi/lo split so the broadcast matmul can run in bf16
                # (two accumulating passes) at full fp32 broadcast accuracy
                rec_hi = sb.tile((16, 512), BF16, tag="rec_hi")
                nc.vector.tensor_copy(rec_hi[:], rec16[:])
                rec_hif = sb.tile((16, 512), F32, tag="rec_hif")
                nc.scalar.activation(rec_hif[:], rec_hi[:], AF.Copy)
                rec_lo = sb.tile((16, 512), BF16, tag="rec_lo")
                nc.vector.tensor_sub(rec_lo[:], rec16[:], rec_hif[:])

            # ---- phase C: normalize + partial out-projection ----
            with tc.tile_pool(name="psC", bufs=1, space="PSUM") as psC:
                for t in range(8):
                    qs = 512 * t
                    bcp = psC.tile((128, 512), F32, tag="bcp", bufs=2)
                    nc.tensor.matmul(
                        bcp[:],
                        S_all[:, 128 * t : 128 * t + 128],
                        rec_hi[:],
                        start=True,
                        stop=False,
                        skip_group_check=True,
                    )
                    nc.tensor.matmul(
                        bcp[:],
                        S_all[:, 128 * t : 128 * t + 128],
                        rec_lo[:],
                        start=False,
                        stop=True,
                        skip_group_check=True,
                    )
                    attnT = sb.tile((128, 512), BF16, tag="attnT", bufs=2)
                    for hl in range(2):
                        nc.vector.tensor_mul(
                            attnT[64 * hl : 64 * hl + 64, :],
                            av_sb[t][0:64, 512 * hl : 512 * hl + 512],
                            bcp[64 * hl : 64 * hl + 64, :],
                        )
                    for mo in range(8):
                        op = psC.tile((128, 512), F32, tag="op", bufs=2)
                        nc.tensor.matmul(
                            op[:],
                            wout_t[:, 128 * mo : 128 * mo + 128],
                            attnT[:],
                            start=True,
                            stop=True,
                            skip_group_check=True,
                        )
                        osb = sb.tile((128, 512), F16, tag="osb", bufs=4)
                        if mo % 2 == 0:
                            nc.vector.tensor_copy(osb[:], op[:])
                        else:
                            nc.scalar.activation(osb[:], op[:], AF.Copy)
                        eng = nc.sync if mo % 2 == 0 else nc.gpsimd
                        eng.dma_start(
                            outp_d[128 * mo : 128 * mo + 128, qs : qs + 512],
                            osb[:],
                        )
    _split_excess_waits(nc)
    return nc


def kernel(**inputs):
    global LAST_EXEC_NS, LAST_MEAN_NS
    x = np.asarray(inputs["x"], np.float32)
    Wqkv = np.asarray(inputs["W_qkv"], np.float32)
    bqkv = np.asarray(inputs["b_qkv"], np.float32)
    Wout = np.asarray(inputs["W_out"], np.float32)
    bout = np.asarray(inputs["b_out"], np.float32)

    xt = np.ascontiguousarray(x.reshape(T, 1024).T).astype(BF)
    kk = np.arange(128)[:, None]
    qq = np.arange(128)[None, :]
    tri = (qq >= kk).astype(BF)
    ident = np.eye(128).astype(BF)
    sel = np.zeros((16, 1024), BF)
    for t in range(8):
        sel[2 * t, 128 * t : 128 * t + 64] = 1.0
        sel[2 * t + 1, 128 * t + 64 : 128 * t + 128] = 1.0

    in_maps = []
    for c in range(8):
        s = 128 * c
        wq = np.ascontiguousarray(
            np.concatenate(
                [
                    Wqkv[:, s : s + 128],
                    Wqkv[:, 1024 + s : 1024 + s + 128],
                    Wqkv[:, 2048 + s : 2048 + s + 128],
                ],
                axis=1,
            )
        ).astype(BF)
        in_maps.append(
            {
                "xt": xt,
                "wqkv": wq,
                "bq": np.ascontiguousarray(
                    bqkv[s : s + 128].reshape(128, 1)
                ).astype(np.float32),
                "wout": np.ascontiguousarray(Wout[s : s + 128, :]).astype(BF),
                "tri": tri,
                "sel": sel,
                "ident": ident,
            }
        )

    nc = _build()
    res = run_bass_kernel_spmd(nc, in_maps, list(range(8)), trace=TRACE)
    LAST_EXEC_NS = res.exec_time_ns
    LAST_MEAN_NS = res.mean_exec_time_ns

    total = np.zeros((1024, T), np.float32)
    for c in range(8):
        total += np.asarray(res.results[c]["outp"]).astype(np.float32)
    beff = (
        bout.astype(np.float64) + bqkv[2048:].astype(np.float64) @ Wout.astype(np.float64)
    ).astype(np.float32)
    out = total.T.reshape(2, 2048, 1024) + beff
    return out.astype(np.float32)



# revision 46
# speedup vs baseline: 1.2149x; 1.2149x over previous
"""MultiHeadAttention (B=2,N=2048,C=1024,H=16,Dk=64) on 8 TRN2 cores.

Head-tensor-parallel: core c owns heads {2c,2c+1} for both batches.
Device computes qkv^T = Wqkv_s^T @ x^T, causal softmax(q k^T/8) @ v, and the
partial out-projection (rows 128c:128c+128 of W_out); host sums the 8
partials (the "all-reduce"), transposes, and adds the fused bias.
b_k drops (softmax shift invariance); b_v folds into the output bias.

Single fused schedule keeps the tensor engine continuously busy (the PE
p-state ramps to 2.4GHz only after ~3us of uninterrupted execution and
drops back on ~1us stalls):
  - x^T streams in 512-token column slices; QKV accumulates per-slice in
    2 PSUM banks (kc-inner), so compute starts ~2us in behind a short
    warm-up matmul burst.
  - v is relaid out token-major by SBUF->SBUF DMA transpose (no PE).
  - attention blocks for batch b interleave with QKV slices of batch b+1
    and with the out-projection of batch b-1, as PE filler so score->exp->
    weighted-sum dependencies never leave the PE idle.
  - exp runs on ACT; PSUM evacuations are spread over DVE/ACT/Pool.
  - softmax reciprocal is bf16 (no Dekker split), done once per batch.
"""
import sys

sys.path.insert(0, "/opt/trn_rl_repo")
import numpy as np
import ml_dtypes
import concourse.bass as bass
import concourse.mybir as mybir
from concourse.bass_utils import run_bass_kernel_spmd
from concourse.tile import TileContext

F32 = mybir.dt.float32
F16 = mybir.dt.float16
BF16 = mybir.dt.bfloat16
AF = mybir.ActivationFunctionType
BF = ml_dtypes.bfloat16

T = 4096  # total tokens (2 batches x 2048)
TRACE = False
LAST_EXEC_NS = None
LAST_MEAN_NS = None

_MAX_WAITS = 1  # this neuronxcc build rejects instructions with more sem waits


def _split_excess_waits(nc, limit=_MAX_WAITS):
    """Move excess sem waits onto same-engine nops inserted just before the
    over-subscribed instruction (waits-before-inst on the same queue is
    semantically identical)."""
    ifaces = [nc.tensor, nc.scalar, nc.vector, nc.gpsimd, nc.sync]
    eng_map = {iface.engine: iface for iface in ifaces}
    f = nc.m.functions[0]
    for bb in list(f.blocks):
        il = bb.instructions
        i = 0
        while i < len(il):
            ins = il[i]
            si = ins.sync_info
            waits = list(si.on_wait) if si is not None else []
            if len(waits) > limit:
                keep = waits[-limit:]
                rest = waits[:-limit]
                ins.sync_info = mybir.SyncInfo(
                    on_wait=keep, on_update=list(si.on_update)
                )
                nops = []
                for k in range(0, len(rest), limit):
                    nop = eng_map[ins.engine].nop(nofuse=True)
                    nop.ins.sync_info = mybir.SyncInfo(
                        on_wait=rest[k : k + limit], on_update=[]
                    )
                    nops.append(nop.ins)
                for ni in nops:
                    for bb2 in list(f.blocks):
                        try:
                            bb2.instructions.remove(ni)
                            break
                        except ValueError:
                            pass
                for off, ni in enumerate(nops):
                    il.insert(i + off, ni)
                i += len(nops)
            i += 1


def _chunk2(ap, stride, n, w):
    """3D AP: [partition, [stride, n], [1, w]] over an existing 2D slice."""
    return bass.AP(
        ap.tensor, ap.offset, [[ap.ap[0][0], ap.ap[0][1]], [stride, n], [1, w]]
    )


def _build():
    nc = bass.Bass("TRN2", target_bir_lowering=False, debug=False, num_devices=8)
    # xt2 row p holds [n=0: kc=0..7 x 512 | n=1: ...] so one DMA lands a full
    # 512-token slice across all eight 128-row contraction chunks.
    xt_d = nc.declare_dram_parameter("xt2", (128, 32768), BF16, isOutput=False)
    wq_d = nc.declare_dram_parameter("wq2", (128, 3072), BF16, isOutput=False)
    bq_d = nc.declare_dram_parameter("bq", (128, 1), F32, isOutput=False)
    # misc = [wout (1024) | tri|tri (256) | sel (1024, rows 0:8)]
    misc_d = nc.declare_dram_parameter("misc", (128, 2304), BF16, isOutput=False)
    outp_d = nc.declare_dram_parameter("outp", (1024, T), F16, isOutput=True)

    with TileContext(nc) as tc:
        sb_cm = tc.tile_pool(name="sb", bufs=1)
        sb = sb_cm.__enter__()

        # ---- persistent tiles ----
        wq_cat = sb.tile((128, 3072), BF16, tag="wq")
        xt_cat = sb.tile((128, 32768), BF16, tag="xt")
        bq_t = sb.tile((128, 1), F32, tag="bq")
        misc_t = sb.tile((128, 2304), BF16, tag="misc")
        wout_t = misc_t[:, 0:1024]
        tri2_t = misc_t[:, 1024:1280]
        sel_t = misc_t[0:8, 1280:2304]
        wrm_t = sb.tile((128, 512), BF16, tag="wrm")

        q_T = sb.tile((128, T), BF16, tag="q_T")
        k_T = sb.tile((128, T), BF16, tag="k_T")
        v_T = sb.tile((128, T), BF16, tag="v_T")
        vext = [
            sb.tile((128, 2080), BF16, tag=f"vext{b}", name=f"vext{b}")
            for b in range(2)
        ]
        es_diag = [
            sb.tile((128, 1024), BF16, tag=f"esd{r}", name=f"esd{r}") for r in range(4)
        ]
        av_sb = [
            sb.tile((65, 1024), F32, tag=f"avsb{t}", name=f"avsb{t}") for t in range(8)
        ]
        # per-block softmax denominators: reciprocal runs right after block t
        # so its out-projection can interleave with the next attention block
        den_tt = [
            sb.tile((2, 512), F32, tag=f"dent{t}", name=f"dent{t}") for t in range(8)
        ]
        recb_t = [
            sb.tile((2, 512), BF16, tag=f"recbt{t}", name=f"recbt{t}")
            for t in range(8)
        ]

        # ---- input DMAs ----
        # all DMAs ride the two hardware DGE queues (sync + scalar engines);
        # gpsimd DMA is software DGE and costs ~800ns of Pool time per call,
        # which would stall the causal-mask muls that live on Pool. The HWDGE
        # front-end is a single shared device at ~630ns per DMA, so inputs
        # are host-packed down to 11 transfers.
        nc.sync.dma_start(
            bass.AP(
                xt_cat.tensor,
                xt_cat.offset,
                [[xt_cat.ap[0][0], 128], [4096, 8], [1, 512]],
            ),
            xt_d[:, 0:4096],
        )
        # weights split m-major: the m=0 slab + bias unblock the first QKV
        # unit ~2us earlier than one 768KB transfer would
        nc.scalar.dma_start(wq_cat[:, 0:1024], wq_d[:, 0:1024])
        nc.scalar.dma_start(bq_t[:], bq_d[:, :])
        nc.scalar.dma_start(wq_cat[:, 1024:3072], wq_d[:, 1024:3072])
        nc.scalar.dma_start(misc_t[:], misc_d[:, :])
        for n in range(1, 8):
            nc.sync.dma_start(
                bass.AP(
                    xt_cat.tensor,
                    xt_cat.offset + 512 * n,
                    [[xt_cat.ap[0][0], 128], [4096, 8], [1, 512]],
                ),
                xt_d[:, 4096 * n : 4096 * n + 4096],
            )

        # ones columns of vext feed the softmax denominator row of av
        nc.vector.memset(wrm_t[:], 0.0)
        nc.vector.memset(vext[0][:], 1.0)
        nc.vector.memset(vext[1][:], 1.0)
        # fully-masked key columns of the diagonal es tiles are zeroed once;
        # exp writes only the unmasked band afterwards
        for r in range(1, 4):
            nc.gpsimd.memset(es_diag[r][:, 0 : 128 * r], 0.0)
            nc.gpsimd.memset(es_diag[r][:, 512 : 512 + 128 * r], 0.0)

        # PSUM pools form a stack; psA (on top) is released mid-kernel to
        # make room for the out-projection pool psC.
        psB_cm = tc.tile_pool(name="psB", bufs=1, space="PSUM")
        psB = psB_cm.__enter__()
        psA_cm = tc.tile_pool(name="psA", bufs=1, space="PSUM")
        psA = psA_cm.__enter__()

        # ---- warm-up: ramp the PE p-state while the first x slice lands.
        # Runs on a zeroed SBUF tile so it depends on no DMA.
        for w in range(18):
            ch = psA.tile((128, 512), F32, tag="ch", bufs=2, name="chw")
            nc.tensor.matmul(
                ch[:],
                wrm_t[:, 0:128],
                wrm_t[:],
                start=True,
                stop=True,
                skip_group_check=True,
            )

        # ---- QKV projection: one (slice, m) unit = 8 accumulating matmuls ----
        qkv_dst = [q_T, k_T, v_T]

        def emit_A_unit(n, m):
            ch = psA.tile((128, 512), F32, tag="ch", bufs=2, name="ch")
            for kc in range(8):
                nc.tensor.matmul(
                    ch[:],
                    wq_cat[:, 384 * kc + 128 * m : 384 * kc + 128 * m + 128],
                    xt_cat[:, 4096 * kc + 512 * n : 4096 * kc + 512 * n + 512],
                    start=(kc == 0),
                    stop=(kc == 7),
                    skip_group_check=True,
                )
            # q/v evacuations on DVE, k on ACT: three DVE evacs per unit
            # (2.3us) against 1.7us of PE work starves DVE early, while ACT
            # can absorb one copy per unit without the exp chain slipping
            o = qkv_dst[m][:, 512 * n : 512 * n + 512]
            if m == 0:
                nc.vector.tensor_scalar_add(o, ch[:], bq_t[:, 0:1])
            elif m == 1:
                nc.scalar.activation(o, ch[:], AF.Copy)
            else:
                nc.vector.tensor_copy(o, ch[:])

        def emit_V_dma(n):
            # DMA-transpose each 128-token block into a scratch tile (the
            # xbar path only supports plain 2D outputs), then one strided
            # DVE copy scatters both head-halves into vext's 65-wide slots.
            b, i = divmod(n, 4)
            for u in range(4):
                jj = 4 * i + u
                vtr = sb.tile((128, 128), BF16, tag="vtr", bufs=2, name="vtr")
                nc.sync.dma_start_transpose(
                    vtr[:], v_T[:, 512 * n + 128 * u : 512 * n + 128 * u + 128]
                )
                ovp = vext[b][:, 65 * jj : 65 * jj + 64]
                o_ap = _chunk2(ovp, 1040, 2, 64)
                i_ap = _chunk2(vtr[:, 0:128], 64, 2, 64)
                # DVE, not Pool: the Pool queue carries the causal-mask muls
                # which sit on the exp->AV critical path
                nc.vector.tensor_copy(o_ap, i_ap)

        # filler queue: units of ready PE work fed into attention bubbles.
        # entries are (slice_id_or_None, emit_fn); A-units carry their slice
        # id so attention block i can force slices <= i out first.
        filler = []

        def take_filler(k):
            for _ in range(k):
                if filler:
                    filler.pop(0)[1]()

        def ensure_slices(n):
            while filler and filler[0][0] is not None and filler[0][0] <= n:
                filler.pop(0)[1]()

        def drain_A_units():
            while filler and filler[0][0] is not None:
                filler.pop(0)[1]()

        def drain_filler():
            while filler:
                filler.pop(0)[1]()

        # ---- attention scores+exp for one 128-key block ----
        def emit_B_scores(b, i, jj):
            sps = psB.tile((128, 1024), F32, tag="sps", bufs=2, name="sps")
            ks = 2048 * b + 128 * jj
            qs = 2048 * b + 512 * i
            for hl in range(2):
                nc.tensor.matmul(
                    sps[:, 512 * hl : 512 * hl + 512],
                    k_T[64 * hl : 64 * hl + 64, ks : ks + 128],
                    q_T[64 * hl : 64 * hl + 64, qs : qs + 512],
                    start=True,
                    stop=True,
                    skip_group_check=True,
                )
            r = jj - 4 * i
            if r < 0:
                es = sb.tile((128, 1024), BF16, tag="es", bufs=3, name="es")
                nc.scalar.activation(es[:], sps[:], AF.Exp, scale=0.125)
            else:
                es = es_diag[r]
                if r == 0:
                    nc.scalar.activation(es[:], sps[:], AF.Exp, scale=0.125)
                else:
                    w = 512 - 128 * r
                    o_ap = _chunk2(es[:, 128 * r : 128 * r + w], 512, 2, w)
                    i_ap = _chunk2(sps[:, 128 * r : 128 * r + w], 512, 2, w)
                    nc.scalar.activation(o_ap, i_ap, AF.Exp, scale=0.125)
                # causal triangle on the diagonal 128 columns, both heads.
                # SBUF-only op -> Pool engine, keeping DVE free for PSUM evacs
                e_ap = _chunk2(es[:, 128 * r : 128 * r + 128], 512, 2, 128)
                t_ap = _chunk2(tri2_t[:, 0:128], 128, 2, 128)
                nc.gpsimd.tensor_mul(e_ap, e_ap, t_ap)
            return es

        def emit_B_av(b, av, jj, es, last):
            for hl in range(2):
                c = 65 * (16 * hl + jj)
                nc.tensor.matmul(
                    av[hl][:],
                    vext[b][:, c : c + 65],
                    es[:, 512 * hl : 512 * hl + 512],
                    start=(jj == 0),
                    stop=last,
                    skip_group_check=True,
                )

        def pump_B_step(b, i, fill_per_jj=1, fill_floor=0):
            t = 4 * b + i
            nj = 4 * i + 4
            av = [
                psB.tile((65, 512), F32, tag=f"av{hl}", bufs=1, name=f"av{hl}")
                for hl in range(2)
            ]
            pend = None
            for jj in range(nj):
                es = emit_B_scores(b, i, jj)
                if pend is not None:
                    emit_B_av(b, av, pend[0], pend[1], last=False)
                if fill_per_jj and len(filler) > fill_floor:
                    k = fill_per_jj + (1 if len(filler) > 24 else 0)
                    take_filler(k)
                pend = (jj, es)
            emit_B_av(b, av, pend[0], pend[1], last=True)
            # evacuate (DVE; ACT would delay the next block's exps and Pool
            # has no PSUM port)
            nc.vector.tensor_copy(av_sb[t][:, 0:512], av[0][:])
            nc.vector.tensor_copy(av_sb[t][:, 512:1024], av[1][:])
            # denominator DMA + this block's reciprocal (bf16, no Dekker)
            dsl = av_sb[t][64:65, 0:1024]
            nc.sync.dma_start(den_tt[t][:, :], _chunk2(dsl, 512, 2, 512))
            recf = sb.tile((2, 512), F32, tag="recf", bufs=2, name="recf")
            nc.vector.reciprocal(recf[:], den_tt[t][:])
            nc.gpsimd.tensor_copy(recb_t[t][:], recf[:])

        # ---- out-projection units for one query block t ----
        OSB_BUFS = 4

        def emit_C_bcast(t, pool, tag, bufs):
            bcp = pool.tile((128, 512), F32, tag=tag, bufs=bufs, name="bcp")
            nc.tensor.matmul(
                bcp[:],
                sel_t[0:2, 128 * t : 128 * t + 128],
                recb_t[t][:],
                start=True,
                stop=True,
                skip_group_check=True,
            )
            attnT = sb.tile((128, 512), BF16, tag="attnT", bufs=2, name="attnT")
            for hl in range(2):
                nc.vector.tensor_mul(
                    attnT[64 * hl : 64 * hl + 64, :],
                    av_sb[t][0:64, 512 * hl : 512 * hl + 512],
                    bcp[64 * hl : 64 * hl + 64, :],
                )
            return attnT

        osb_cur = [None]

        def emit_C_mo(t, mo, attnT, pool, tag, bufs, evac_engs):
            op = pool.tile((128, 512), F32, tag=tag, bufs=bufs, name="op")
            nc.tensor.matmul(
                op[:],
                wout_t[:, 128 * mo : 128 * mo + 128],
                attnT[:],
                start=True,
                stop=True,
                skip_group_check=True,
            )
            # evacuate mo-pairs into one (128,1024) staging tile; a single
            # DMA then scatters 256 output rows (keeps the HWDGE count low)
            if mo % 2 == 0:
                osb_cur[0] = sb.tile(
                    (128, 1024), F16, tag="osb", bufs=OSB_BUFS, name="osb"
                )
            osb = osb_cur[0]
            half = mo % 2
            eng = evac_engs[mo % len(evac_engs)]
            dst = osb[:, 512 * half : 512 * half + 512]
            if eng == "v":
                nc.vector.tensor_copy(dst, op[:])
            else:
                nc.scalar.activation(dst, op[:], AF.Copy)
            if half == 1:
                src = _chunk2(osb[:, 0:1024], 512, 2, 512)
                base = outp_d[0:128, 0:512]
                doff = base.offset + 128 * (mo - 1) * T + 512 * t
                dap = bass.AP(
                    base.tensor, doff, [[T, 128], [128 * T, 2], [1, 512]]
                )
                deng = nc.sync if mo % 4 == 1 else nc.scalar
                deng.dma_start(dap, src)

        # =====================  schedule  =====================
        # QKV slice 0 up front; slices 1..7 and each finished block's
        # out-projection are queued as PE filler consumed inside the
        # attention pumps. The out-projection of block t shares the psA
        # "ch" PSUM ring with the QKV accumulation units.
        c_attnT = {}

        def emit_C_unit(u):
            if u[0] == "b":
                c_attnT[u[1]] = emit_C_bcast(u[1], psA, "ch", 2)
            else:
                _, t, mo = u
                emit_C_mo(t, mo, c_attnT[t], psA, "ch", 2, ["v"])

        for m in range(3):
            emit_A_unit(0, m)
        emit_V_dma(0)
        for n in range(1, 8):
            for m in range(3):
                filler.append((n, lambda n=n, m=m: emit_A_unit(n, m)))
            filler.append((n, lambda n=n: emit_V_dma(n)))

        for b in range(2):
            for i in range(4):
                t = 4 * b + i
                ensure_slices(4 * b + i)
                # hold ~8 units back at the end to cover the final recip +
                # pool-transition window before block 7's out-projection
                pump_B_step(b, i, fill_per_jj=1, fill_floor=8 if t == 7 else 0)
                if t < 7:
                    filler.append((None, lambda t=t: emit_C_unit(("b", t))))
                    for mo in range(8):
                        filler.append(
                            (None, lambda t=t, mo=mo: emit_C_unit(("m", t, mo)))
                        )
        drain_filler()

        psA_cm.__exit__(None, None, None)
        psB_cm.__exit__(None, None, None)
        psC2_cm = tc.tile_pool(name="psC2", bufs=1, space="PSUM")
        psC2 = psC2_cm.__enter__()

        # final block's out-projection with deep rings (nothing left to hide
        # behind, so give the evacuation maximum pipeline depth)
        attnT7 = emit_C_bcast(7, psC2, "bcp2", 2)
        for mo in range(8):
            emit_C_mo(7, mo, attnT7, psC2, "op2", 4, ["s", "v"])

        psC2_cm.__exit__(None, None, None)
        sb_cm.__exit__(None, None, None)

    _split_excess_waits(nc)
    return nc


def _prepare_inmaps(inputs):
    x = np.asarray(inputs["x"], np.float32)
    Wqkv = np.asarray(inputs["W_qkv"], np.float32)
    bqkv = np.asarray(inputs["b_qkv"], np.float32)
    Wout = np.asarray(inputs["W_out"], np.float32)

    # xt2[p, 4096*n + 512*kc + j] = x^T[128*kc + p, 512*n + j]
    xt = x.reshape(T, 1024).T.reshape(8, 128, 8, 512)  # (kc, p, n, j)
    xt2 = np.ascontiguousarray(xt.transpose(1, 2, 0, 3).reshape(128, 32768)).astype(
        BF
    )

    kk = np.arange(128)[:, None]
    qq = np.arange(128)[None, :]
    tri = (qq >= kk).astype(np.float32)
    tri2 = np.concatenate([tri, tri], axis=1)
    sel = np.zeros((128, 1024), np.float32)
    for t in range(8):
        sel[0, 128 * t : 128 * t + 64] = 1.0
        sel[1, 128 * t + 64 : 128 * t + 128] = 1.0

    in_maps = []
    for c in range(8):
        s = 128 * c
        wq = np.concatenate(
            [
                Wqkv[:, s : s + 128],
                Wqkv[:, 1024 + s : 1024 + s + 128],
                Wqkv[:, 2048 + s : 2048 + s + 128],
            ],
            axis=1,
        )  # (1024, 384)
        # wq2[p, 384*kc + col] = wq[128*kc + p, col]
        wq2 = np.ascontiguousarray(
            wq.reshape(8, 128, 384).transpose(1, 0, 2).reshape(128, 3072)
        ).astype(BF)
        misc = np.ascontiguousarray(
            np.concatenate([Wout[s : s + 128, :], tri2, sel], axis=1)
        ).astype(BF)
        in_maps.append(
            {
                "xt2": xt2,
                "wq2": wq2,
                "bq": np.ascontiguousarray(
                    bqkv[s : s + 128].reshape(128, 1)
                ).astype(np.float32),
                "misc": misc,
            }
        )
    return in_maps


def _postprocess(results, inputs):
    bqkv = np.asarray(inputs["b_qkv"], np.float32)
    Wout = np.asarray(inputs["W_out"], np.float32)
    bout = np.asarray(inputs["b_out"], np.float32)
    total = np.zeros((1024, T), np.float32)
    for c in range(8):
        total += np.asarray(results[c]["outp"]).astype(np.float32)
    beff = (
        bout.astype(np.float64)
        + bqkv[2048:].astype(np.float64) @ Wout.astype(np.float64)
    ).astype(np.float32)
    out = total.T.reshape(2, 2048, 1024) + beff
    return out.astype(np.float32)


def _numpy_partials(inputs, cores):
    """Per-core expected partial outp (float32, [1024, T]) replicating the
    device math in numpy (for subset sim checks)."""
    x = np.asarray(inputs["x"], np.float64)
    Wqkv = np.asarray(inputs["W_qkv"], np.float64)
    bqkv = np.asarray(inputs["b_qkv"], np.float64)
    Wout = np.asarray(inputs["W_out"], np.float64)
    xt = x.reshape(T, 1024)
    out = {}
    for c in cores:
        s = 128 * c
        q = xt @ Wqkv[:, s : s + 128] + bqkv[s : s + 128]
        k = xt @ Wqkv[:, 1024 + s : 1024 + s + 128]
        v = xt @ Wqkv[:, 2048 + s : 2048 + s + 128]
        part = np.zeros((T, 1024))
        for b in range(2):
            sl = slice(2048 * b, 2048 * b + 2048)
            for h in range(2):
                hs = slice(64 * h, 64 * h + 64)
                sc = q[sl, hs] @ k[sl, hs].T / 8.0
                mask = np.tril(np.ones((2048, 2048), dtype=bool))
                sc = np.where(mask, sc, -np.inf)
                w = np.exp(sc - sc.max(axis=1, keepdims=True))
                w /= w.sum(axis=1, keepdims=True)
                part[sl] += (w @ v[sl, hs]) @ Wout[s + 64 * h : s + 64 * h + 64, :]
        out[c] = part.T.astype(np.float32)
    return out


def kernel(**inputs):
    global LAST_EXEC_NS, LAST_MEAN_NS
    in_maps = _prepare_inmaps(inputs)
    nc = _build()
    res = run_bass_kernel_spmd(nc, in_maps, list(range(8)), trace=TRACE)
    LAST_EXEC_NS = res.exec_time_ns
    LAST_MEAN_NS = res.mean_exec_time_ns
    return _postprocess(res.results, inputs)


# revision 61
# speedup vs baseline: 1.2517x; 1.0303x over previous
"""MultiHeadAttention (B=2,N=2048,C=1024,H=16,Dk=64) on 8 TRN2 cores.

Head-tensor-parallel: core c owns heads {2c,2c+1} for both batches.
Device computes qkv^T = Wqkv_s^T @ x^T, causal softmax(q k^T/8) @ v, and the
partial out-projection (rows 128c:128c+128 of W_out); host sums the 8
partials (the "all-reduce"), transposes, and adds the fused bias.
b_k drops (softmax shift invariance); b_v folds into the output bias.

Single fused schedule keeps the tensor engine continuously busy (the PE
p-state ramps to 2.4GHz only after ~3us of uninterrupted execution and
drops back on ~1us stalls):
  - x^T streams in 512-token column slices; QKV accumulates per-slice in
    2 PSUM banks (kc-inner), so compute starts ~2us in behind a short
    warm-up matmul burst.
  - v is relaid out token-major by SBUF->SBUF DMA transpose (no PE).
  - attention blocks for batch b interleave with QKV slices of batch b+1
    and with the out-projection of batch b-1, as PE filler so score->exp->
    weighted-sum dependencies never leave the PE idle.
  - exp runs on ACT; PSUM evacuations are spread over DVE/ACT/Pool.
  - softmax reciprocal is bf16 (no Dekker split), done once per batch.
"""
import sys

sys.path.insert(0, "/opt/trn_rl_repo")
import numpy as np
import ml_dtypes
import concourse.bass as bass
import concourse.mybir as mybir
from concourse.bass_utils import run_bass_kernel_spmd
from concourse.tile import TileContext

F32 = mybir.dt.float32
F16 = mybir.dt.float16
BF16 = mybir.dt.bfloat16
AF = mybir.ActivationFunctionType
BF = ml_dtypes.bfloat16

T = 4096  # total tokens (2 batches x 2048)
TRACE = False
LAST_EXEC_NS = None
LAST_MEAN_NS = None

_MAX_WAITS = 1  # this neuronxcc build rejects instructions with more sem waits


def _split_excess_waits(nc, limit=_MAX_WAITS):
    """Move excess sem waits onto same-engine nops inserted just before the
    over-subscribed instruction (waits-before-inst on the same queue is
    semantically identical)."""
    ifaces = [nc.tensor, nc.scalar, nc.vector, nc.gpsimd, nc.sync]
    eng_map = {iface.engine: iface for iface in ifaces}
    f = nc.m.functions[0]
    for bb in list(f.blocks):
        il = bb.instructions
        i = 0
        while i < len(il):
            ins = il[i]
            si = ins.sync_info
            waits = list(si.on_wait) if si is not None else []
            if len(waits) > limit:
                keep = waits[-limit:]
                rest = waits[:-limit]
                ins.sync_info = mybir.SyncInfo(
                    on_wait=keep, on_update=list(si.on_update)
                )
                nops = []
                for k in range(0, len(rest), limit):
                    nop = eng_map[ins.engine].nop(nofuse=True)
                    nop.ins.sync_info = mybir.SyncInfo(
                        on_wait=rest[k : k + limit], on_update=[]
                    )
                    nops.append(nop.ins)
                for ni in nops:
                    for bb2 in list(f.blocks):
                        try:
                            bb2.instructions.remove(ni)
                            break
                        except ValueError:
                            pass
                for off, ni in enumerate(nops):
                    il.insert(i + off, ni)
                i += len(nops)
            i += 1


def _chunk2(ap, stride, n, w):
    """3D AP: [partition, [stride, n], [1, w]] over an existing 2D slice."""
    return bass.AP(
        ap.tensor, ap.offset, [[ap.ap[0][0], ap.ap[0][1]], [stride, n], [1, w]]
    )


def _build():
    nc = bass.Bass("TRN2", target_bir_lowering=False, debug=False, num_devices=8)
    # xt2 row p holds [n=0: kc=0..7 x 512 | n=1: ...] so one DMA lands a full
    # 512-token slice across all eight 128-row contraction chunks.
    xt_d = nc.declare_dram_parameter("xt2", (128, 32768), BF16, isOutput=False)
    wq_d = nc.declare_dram_parameter("wq2", (128, 3072), BF16, isOutput=False)
    bq_d = nc.declare_dram_parameter("bq", (128, 1), F32, isOutput=False)
    # misc = [wout (1024) | tri|tri (256)]
    misc_d = nc.declare_dram_parameter("misc", (128, 1280), BF16, isOutput=False)
    outp_d = nc.declare_dram_parameter("outp", (1024, T), F16, isOutput=True)

    with TileContext(nc) as tc:
        sb_cm = tc.tile_pool(name="sb", bufs=1)
        sb = sb_cm.__enter__()

        # ---- persistent tiles ----
        wq_cat = sb.tile((128, 3072), BF16, tag="wq")
        xt_cat = sb.tile((128, 32768), BF16, tag="xt")
        bq_t = sb.tile((128, 1), F32, tag="bq")
        misc_t = sb.tile((128, 1280), BF16, tag="misc")
        wout_t = misc_t[:, 0:1024]
        tri2_t = misc_t[:, 1024:1280]
        wrm_t = sb.tile((128, 512), BF16, tag="wrm")
        ones1_t = sb.tile((1, 64), BF16, tag="ones1")

        q_T = sb.tile((128, T), BF16, tag="q_T")
        k_T = sb.tile((128, T), BF16, tag="k_T")
        v_T = sb.tile((128, T), BF16, tag="v_T")
        vext = [
            sb.tile((128, 2080), BF16, tag=f"vext{b}", name=f"vext{b}")
            for b in range(2)
        ]
        # double-buffered per r: block i+1's exp must not wait for block i's
        # AV to release the same diagonal tile
        es_diag = [
            [
                sb.tile((128, 1024), BF16, tag=f"esd{p}{r}", name=f"esd{p}{r}")
                for r in range(4)
            ]
            for p in range(2)
        ]
        av_sb = [
            sb.tile((65, 1024), F32, tag=f"avsb{t}", name=f"avsb{t}") for t in range(8)
        ]
        # per-block softmax denominator reciprocals, computed on ACT as
        # exp(-ln(den)) straight out of the av PSUM row (the ln/exp/copy
        # functions share one activation table: no table reloads)
        recb_t = [
            sb.tile((1, 1024), BF16, tag=f"recbt{t}", name=f"recbt{t}")
            for t in range(8)
        ]

        # ---- input DMAs ----
        # all DMAs ride the two hardware DGE queues (sync + scalar engines);
        # gpsimd DMA is software DGE and costs ~800ns of Pool time per call,
        # which would stall the causal-mask muls that live on Pool. The HWDGE
        # front-end is a single shared device at ~630ns per DMA, so inputs
        # are host-packed down to 11 transfers.
        nc.sync.dma_start(
            bass.AP(
                xt_cat.tensor,
                xt_cat.offset,
                [[xt_cat.ap[0][0], 128], [4096, 8], [1, 512]],
            ),
            xt_d[:, 0:4096],
        )
        nc.scalar.dma_start(wq_cat[:], wq_d[:, :])
        nc.scalar.dma_start(bq_t[:], bq_d[:, :])
        nc.scalar.dma_start(misc_t[:], misc_d[:, :])
        for n in range(1, 8):
            nc.sync.dma_start(
                bass.AP(
                    xt_cat.tensor,
                    xt_cat.offset + 512 * n,
                    [[xt_cat.ap[0][0], 128], [4096, 8], [1, 512]],
                ),
                xt_d[:, 4096 * n : 4096 * n + 4096],
            )

        # ones columns of vext feed the softmax denominator row of av
        nc.vector.memset(wrm_t[:], 0.0)
        nc.vector.memset(ones1_t[:], 1.0)
        nc.vector.memset(vext[0][:], 1.0)
        nc.vector.memset(vext[1][:], 1.0)
        # fully-masked key columns of the diagonal es tiles are zeroed once;
        # exp writes only the unmasked band afterwards
        for p in range(2):
            for r in range(1, 4):
                nc.gpsimd.memset(es_diag[p][r][:, 0 : 128 * r], 0.0)
                nc.gpsimd.memset(es_diag[p][r][:, 512 : 512 + 128 * r], 0.0)

        # PSUM pools form a stack; psA (on top) is released mid-kernel to
        # make room for the out-projection pool psC.
        psB_cm = tc.tile_pool(name="psB", bufs=1, space="PSUM")
        psB = psB_cm.__enter__()
        psA_cm = tc.tile_pool(name="psA", bufs=1, space="PSUM")
        psA = psA_cm.__enter__()

        # ---- warm-up: ramp the PE p-state while the first x slice lands.
        # Runs on a zeroed SBUF tile so it depends on no DMA.
        for w in range(18):
            ch = psA.tile((128, 512), F32, tag="ch", bufs=2, name="chw")
            nc.tensor.matmul(
                ch[:],
                wrm_t[:, 0:128],
                wrm_t[:],
                start=True,
                stop=True,
                skip_group_check=True,
            )

        # ---- QKV projection: one (slice, m) unit = 8 accumulating matmuls ----
        qkv_dst = [q_T, k_T, v_T]

        def emit_A_unit(n, m):
            ch = psA.tile((128, 512), F32, tag="ch", bufs=2, name="ch")
            for kc in range(8):
                nc.tensor.matmul(
                    ch[:],
                    wq_cat[:, 384 * kc + 128 * m : 384 * kc + 128 * m + 128],
                    xt_cat[:, 4096 * kc + 512 * n : 4096 * kc + 512 * n + 512],
                    start=(kc == 0),
                    stop=(kc == 7),
                    skip_group_check=True,
                )
            # q/v evacuations on DVE, k on ACT: three DVE evacs per unit
            # (2.3us) against 1.7us of PE work starves DVE early, while ACT
            # can absorb one copy per unit without the exp chain slipping
            o = qkv_dst[m][:, 512 * n : 512 * n + 512]
            if m == 0:
                nc.vector.tensor_scalar_add(o, ch[:], bq_t[:, 0:1])
            elif m == 1:
                nc.scalar.activation(o, ch[:], AF.Copy)
            else:
                nc.vector.tensor_copy(o, ch[:])

        def emit_V_dma(n):
            # DMA-transpose each 128-token block into a scratch tile (the
            # xbar path only supports plain 2D outputs), then one strided
            # DVE copy scatters both head-halves into vext's 65-wide slots.
            b, i = divmod(n, 4)
            for u in range(4):
                jj = 4 * i + u
                # scalar queue: the sync queue carries the xt stream and the
                # denominator collects, which are latency-critical
                vtr = sb.tile((128, 128), BF16, tag="vtr", bufs=2, name="vtr")
                nc.scalar.dma_start_transpose(
                    vtr[:], v_T[:, 512 * n + 128 * u : 512 * n + 128 * u + 128]
                )
                ovp = vext[b][:, 65 * jj : 65 * jj + 64]
                o_ap = _chunk2(ovp, 1040, 2, 64)
                i_ap = _chunk2(vtr[:, 0:128], 64, 2, 64)
                # DVE, not Pool: the Pool queue carries the causal-mask muls
                # which sit on the exp->AV critical path
                nc.vector.tensor_copy(o_ap, i_ap)

        # filler queue: units of ready PE work fed into attention bubbles.
        # entries are (slice_id_or_None, emit_fn); A-units carry their slice
        # id so attention block i can force slices <= i out first.
        filler = []

        def take_filler(k):
            for _ in range(k):
                if filler:
                    filler.pop(0)[1]()

        def ensure_slices(n):
            while filler and filler[0][0] is not None and filler[0][0] <= n:
                filler.pop(0)[1]()

        def drain_A_units():
            while filler and filler[0][0] is not None:
                filler.pop(0)[1]()

        def drain_filler():
            while filler:
                filler.pop(0)[1]()

        # ---- attention scores+exp for one 128-key block ----
        def emit_B_scores(b, i, jj):
            sps = psB.tile((128, 1024), F32, tag="sps", bufs=2, name="sps")
            ks = 2048 * b + 128 * jj
            qs = 2048 * b + 512 * i
            for hl in range(2):
                nc.tensor.matmul(
                    sps[:, 512 * hl : 512 * hl + 512],
                    k_T[64 * hl : 64 * hl + 64, ks : ks + 128],
                    q_T[64 * hl : 64 * hl + 64, qs : qs + 512],
                    start=True,
                    stop=True,
                    skip_group_check=True,
                )
            r = jj - 4 * i
            if r < 0:
                es = sb.tile((128, 1024), BF16, tag="es", bufs=4, name="es")
                nc.scalar.activation(es[:], sps[:], AF.Exp, scale=0.125)
            else:
                es = es_diag[(4 * b + i) % 2][r]
                if r == 0:
                    nc.scalar.activation(es[:], sps[:], AF.Exp, scale=0.125)
                else:
                    w = 512 - 128 * r
                    o_ap = _chunk2(es[:, 128 * r : 128 * r + w], 512, 2, w)
                    i_ap = _chunk2(sps[:, 128 * r : 128 * r + w], 512, 2, w)
                    nc.scalar.activation(o_ap, i_ap, AF.Exp, scale=0.125)
                # causal triangle on the diagonal 128 columns, both heads.
                # SBUF-only op -> Pool engine, keeping DVE free for PSUM evacs
                e_ap = _chunk2(es[:, 128 * r : 128 * r + 128], 512, 2, 128)
                t_ap = _chunk2(tri2_t[:, 0:128], 128, 2, 128)
                nc.gpsimd.tensor_mul(e_ap, e_ap, t_ap)
            return es

        def emit_B_av(b, av, jj, es, last):
            for hl in range(2):
                c = 65 * (16 * hl + jj)
                nc.tensor.matmul(
                    av[hl][:],
                    vext[b][:, c : c + 65],
                    es[:, 512 * hl : 512 * hl + 512],
                    start=(jj == 0),
                    stop=last,
                    skip_group_check=True,
                )

        def pump_B_step(b, i, fill_per_jj=1, fill_floor=0):
            t = 4 * b + i
            nj = 4 * i + 4
            av = [
                psB.tile((65, 512), F32, tag=f"av{hl}", bufs=1, name=f"av{hl}")
                for hl in range(2)
            ]
            pend = None
            for jj in range(nj):
                es = emit_B_scores(b, i, jj)
                if pend is not None:
                    emit_B_av(b, av, pend[0], pend[1], last=False)
                if fill_per_jj and len(filler) > fill_floor:
                    k = fill_per_jj + (1 if len(filler) > 24 else 0)
                    take_filler(k)
                pend = (jj, es)
            emit_B_av(b, av, pend[0], pend[1], last=True)
            # denominator reciprocal on ACT: 1/den = exp(-ln(den)), read
            # straight from the av PSUM row. A DVE reciprocal costs 3.3us on
            # hw and a DMA denominator-collect ~2us more of chain latency.
            lt = sb.tile((1, 1024), F32, tag="lt", bufs=2, name="lt")
            for hl in range(2):
                nc.scalar.activation(
                    lt[0:1, 512 * hl : 512 * hl + 512], av[hl][64:65, :], AF.Ln
                )
            nc.scalar.activation(recb_t[t][:], lt[:], AF.Exp, scale=-1.0)
            # evacuate the attention values (DVE; Pool has no PSUM port)
            nc.vector.tensor_copy(av_sb[t][:, 0:512], av[0][:])
            nc.vector.tensor_copy(av_sb[t][:, 512:1024], av[1][:])

        # ---- out-projection units for one query block t ----
        OSB_BUFS = 4

        def emit_C_bcast(t, pool, tag, bufs):
            # two rank-1 matmuls broadcast each head-half's reciprocal row
            # across its 64 partitions (PE quadrant col offset selects rows)
            bcp = pool.tile((128, 512), F32, tag=tag, bufs=bufs, name="bcp")
            for hl in range(2):
                nc.tensor.matmul(
                    bcp[64 * hl : 64 * hl + 64, :],
                    ones1_t[:],
                    recb_t[t][0:1, 512 * hl : 512 * hl + 512],
                    start=True,
                    stop=True,
                    skip_group_check=True,
                    tile_position=(0, 64 * hl),
                )
            attnT = sb.tile((128, 512), BF16, tag="attnT", bufs=2, name="attnT")
            for hl in range(2):
                nc.vector.tensor_mul(
                    attnT[64 * hl : 64 * hl + 64, :],
                    av_sb[t][0:64, 512 * hl : 512 * hl + 512],
                    bcp[64 * hl : 64 * hl + 64, :],
                )
            return attnT

        osb_cur = [None]

        def emit_C_mo(t, mo, attnT, pool, tag, bufs, evac_engs):
            op = pool.tile((128, 512), F32, tag=tag, bufs=bufs, name="op")
            nc.tensor.matmul(
                op[:],
                wout_t[:, 128 * mo : 128 * mo + 128],
                attnT[:],
                start=True,
                stop=True,
                skip_group_check=True,
            )
            # evacuate mo-pairs into one (128,1024) staging tile; a single
            # DMA then scatters 256 output rows (keeps the HWDGE count low)
            if mo % 2 == 0:
                osb_cur[0] = sb.tile(
                    (128, 1024), F16, tag="osb", bufs=OSB_BUFS, name="osb"
                )
            osb = osb_cur[0]
            half = mo % 2
            eng = evac_engs[mo % len(evac_engs)]
            dst = osb[:, 512 * half : 512 * half + 512]
            if eng == "v":
                nc.vector.tensor_copy(dst, op[:])
            else:
                nc.scalar.activation(dst, op[:], AF.Copy)
            if half == 1:
                src = _chunk2(osb[:, 0:1024], 512, 2, 512)
                base = outp_d[0:128, 0:512]
                doff = base.offset + 128 * (mo - 1) * T + 512 * t
                dap = bass.AP(
                    base.tensor, doff, [[T, 128], [128 * T, 2], [1, 512]]
                )
                deng = nc.sync if mo % 4 == 1 else nc.scalar
                deng.dma_start(dap, src)

        # =====================  schedule  =====================
        # QKV slice 0 up front; slices 1..7 and each finished block's
        # out-projection are queued as PE filler consumed inside the
        # attention pumps. The out-projection of block t shares the psA
        # "ch" PSUM ring with the QKV accumulation units.
        c_attnT = {}

        def emit_C_unit(u):
            if u[0] == "b":
                c_attnT[u[1]] = emit_C_bcast(u[1], psA, "ch", 2)
            else:
                _, t, mo = u
                emit_C_mo(t, mo, c_attnT[t], psA, "ch", 2, ["v"])

        for m in range(3):
            emit_A_unit(0, m)
        emit_V_dma(0)
        for n in range(1, 8):
            for m in range(3):
                filler.append((n, lambda n=n, m=m: emit_A_unit(n, m)))
            filler.append((n, lambda n=n: emit_V_dma(n)))

        for b in range(2):
            for i in range(4):
                t = 4 * b + i
                ensure_slices(4 * b + i)
                # hold ~8 units back at the end to cover the final recip +
                # pool-transition window before block 7's out-projection
                pump_B_step(b, i, fill_per_jj=1, fill_floor=10)
                if t < 7:
                    filler.append((None, lambda t=t: emit_C_unit(("b", t))))
                    for mo in range(8):
                        filler.append(
                            (None, lambda t=t, mo=mo: emit_C_unit(("m", t, mo)))
                        )
        drain_filler()

        psA_cm.__exit__(None, None, None)
        psB_cm.__exit__(None, None, None)
        psC2_cm = tc.tile_pool(name="psC2", bufs=1, space="PSUM")
        psC2 = psC2_cm.__enter__()

        # final block's out-projection with deep rings (nothing left to hide
        # behind, so give the evacuation maximum pipeline depth)
        attnT7 = emit_C_bcast(7, psC2, "bcp2", 2)
        for mo in range(8):
            emit_C_mo(7, mo, attnT7, psC2, "op2", 4, ["s", "v"])

        psC2_cm.__exit__(None, None, None)
        sb_cm.__exit__(None, None, None)

    _split_excess_waits(nc)
    return nc


def _prepare_inmaps(inputs):
    x = np.asarray(inputs["x"], np.float32)
    Wqkv = np.asarray(inputs["W_qkv"], np.float32)
    bqkv = np.asarray(inputs["b_qkv"], np.float32)
    Wout = np.asarray(inputs["W_out"], np.float32)

    # xt2[p, 4096*n + 512*kc + j] = x^T[128*kc + p, 512*n + j]
    xt = x.reshape(T, 1024).T.reshape(8, 128, 8, 512)  # (kc, p, n, j)
    xt2 = np.ascontiguousarray(xt.transpose(1, 2, 0, 3).reshape(128, 32768)).astype(
        BF
    )

    kk = np.arange(128)[:, None]
    qq = np.arange(128)[None, :]
    tri = (qq >= kk).astype(np.float32)
    tri2 = np.concatenate([tri, tri], axis=1)

    in_maps = []
    for c in range(8):
        s = 128 * c
        wq = np.concatenate(
            [
                Wqkv[:, s : s + 128],
                Wqkv[:, 1024 + s : 1024 + s + 128],
                Wqkv[:, 2048 + s : 2048 + s + 128],
            ],
            axis=1,
        )  # (1024, 384)
        # wq2[p, 384*kc + col] = wq[128*kc + p, col]
        wq2 = np.ascontiguousarray(
            wq.reshape(8, 128, 384).transpose(1, 0, 2).reshape(128, 3072)
        ).astype(BF)
        misc = np.ascontiguousarray(
            np.concatenate([Wout[s : s + 128, :], tri2], axis=1)
        ).astype(BF)
        in_maps.append(
            {
                "xt2": xt2,
                "wq2": wq2,
                "bq": np.ascontiguousarray(
                    bqkv[s : s + 128].reshape(128, 1)
                ).astype(np.float32),
                "misc": misc,
            }
        )
    return in_maps


def _postprocess(results, inputs):
    bqkv = np.asarray(inputs["b_qkv"], np.float32)
    Wout = np.asarray(inputs["W_out"], np.float32)
    bout = np.asarray(inputs["b_out"], np.float32)
    total = np.zeros((1024, T), np.float32)
    for c in range(8):
        total += np.asarray(results[c]["outp"]).astype(np.float32)
    beff = (
        bout.astype(np.float64)
        + bqkv[2048:].astype(np.float64) @ Wout.astype(np.float64)
    ).astype(np.float32)
    out = total.T.reshape(2, 2048, 1024) + beff
    return out.astype(np.float32)


def _numpy_partials(inputs, cores):
    """Per-core expected partial outp (float32, [1024, T]) replicating the
    device math in numpy (for subset sim checks)."""
    x = np.asarray(inputs["x"], np.float64)
    Wqkv = np.asarray(inputs["W_qkv"], np.float64)
    bqkv = np.asarray(inputs["b_qkv"], np.float64)
    Wout = np.asarray(inputs["W_out"], np.float64)
    xt = x.reshape(T, 1024)
    out = {}
    for c in cores:
        s = 128 * c
        q = xt @ Wqkv[:, s : s + 128] + bqkv[s : s + 128]
        k = xt @ Wqkv[:, 1024 + s : 1024 + s + 128]
        v = xt @ Wqkv[:, 2048 + s : 2048 + s + 128]
        part = np.zeros((T, 1024))
        for b in range(2):
            sl = slice(2048 * b, 2048 * b + 2048)
            for h in range(2):
                hs = slice(64 * h, 64 * h + 64)
                sc = q[sl, hs] @ k[sl, hs].T / 8.0
                mask = np.tril(np.ones((2048, 2048), dtype=bool))
                sc = np.where(mask, sc, -np.inf)
                w = np.exp(sc - sc.max(axis=1, keepdims=True))
                w /= w.sum(axis=1, keepdims=True)
                part[sl] += (w @ v[sl, hs]) @ Wout[s + 64 * h : s + 64 * h + 64, :]
        out[c] = part.T.astype(np.float32)
    return out


def kernel(**inputs):
    global LAST_EXEC_NS, LAST_MEAN_NS
    in_maps = _prepare_inmaps(inputs)
    nc = _build()
    res = run_bass_kernel_spmd(nc, in_maps, list(range(8)), trace=TRACE)
    LAST_EXEC_NS = res.exec_time_ns
    LAST_MEAN_NS = res.mean_exec_time_ns
    return _postprocess(res.results, inputs)


# revision 67
# speedup vs baseline: 1.2686x; 1.0135x over previous
"""MultiHeadAttention (B=2,N=2048,C=1024,H=16,Dk=64) on 8 TRN2 cores.

Head-tensor-parallel: core c owns heads {2c,2c+1} for both batches.
Device computes qkv^T = Wqkv_s^T @ x^T, causal softmax(q k^T/8) @ v, and the
partial out-projection (rows 128c:128c+128 of W_out); host sums the 8
partials (the "all-reduce"), transposes, and adds the fused bias.
b_k drops (softmax shift invariance); b_v folds into the output bias.

Single fused schedule keeps the tensor engine continuously busy (the PE
p-state ramps to 2.4GHz only after ~3us of uninterrupted execution and
drops back on ~1us stalls):
  - x^T streams in 512-token column slices; QKV accumulates per-slice in
    2 PSUM banks (kc-inner), so compute starts ~2us in behind a short
    warm-up matmul burst.
  - v is relaid out token-major by SBUF->SBUF DMA transpose (no PE).
  - attention blocks for batch b interleave with QKV slices of batch b+1
    and with the out-projection of batch b-1, as PE filler so score->exp->
    weighted-sum dependencies never leave the PE idle.
  - exp runs on ACT; PSUM evacuations are spread over DVE/ACT/Pool.
  - softmax reciprocal is bf16 (no Dekker split), done once per batch.
"""
import sys

sys.path.insert(0, "/opt/trn_rl_repo")
import numpy as np
import ml_dtypes
import concourse.bass as bass
import concourse.mybir as mybir
from concourse.bass_utils import run_bass_kernel_spmd
from concourse.tile import TileContext

F32 = mybir.dt.float32
F16 = mybir.dt.float16
BF16 = mybir.dt.bfloat16
AF = mybir.ActivationFunctionType
BF = ml_dtypes.bfloat16

T = 4096  # total tokens (2 batches x 2048)
TRACE = False
LAST_EXEC_NS = None
LAST_MEAN_NS = None

_MAX_WAITS = 1  # this neuronxcc build rejects instructions with more sem waits


def _split_excess_waits(nc, limit=_MAX_WAITS):
    """Move excess sem waits onto same-engine nops inserted just before the
    over-subscribed instruction (waits-before-inst on the same queue is
    semantically identical)."""
    ifaces = [nc.tensor, nc.scalar, nc.vector, nc.gpsimd, nc.sync]
    eng_map = {iface.engine: iface for iface in ifaces}
    f = nc.m.functions[0]
    for bb in list(f.blocks):
        il = bb.instructions
        i = 0
        while i < len(il):
            ins = il[i]
            si = ins.sync_info
            waits = list(si.on_wait) if si is not None else []
            if len(waits) > limit:
                keep = waits[-limit:]
                rest = waits[:-limit]
                ins.sync_info = mybir.SyncInfo(
                    on_wait=keep, on_update=list(si.on_update)
                )
                nops = []
                for k in range(0, len(rest), limit):
                    nop = eng_map[ins.engine].nop(nofuse=True)
                    nop.ins.sync_info = mybir.SyncInfo(
                        on_wait=rest[k : k + limit], on_update=[]
                    )
                    nops.append(nop.ins)
                for ni in nops:
                    for bb2 in list(f.blocks):
                        try:
                            bb2.instructions.remove(ni)
                            break
                        except ValueError:
                            pass
                for off, ni in enumerate(nops):
                    il.insert(i + off, ni)
                i += len(nops)
            i += 1


def _chunk2(ap, stride, n, w):
    """3D AP: [partition, [stride, n], [1, w]] over an existing 2D slice."""
    return bass.AP(
        ap.tensor, ap.offset, [[ap.ap[0][0], ap.ap[0][1]], [stride, n], [1, w]]
    )


def _build():
    nc = bass.Bass("TRN2", target_bir_lowering=False, debug=False, num_devices=8)
    # xt2 row p holds [n=0: kc=0..7 x 512 | n=1: ...] so one DMA lands a full
    # 512-token slice across all eight 128-row contraction chunks.
    xt_d = nc.declare_dram_parameter("xt2", (128, 32768), BF16, isOutput=False)
    wq_d = nc.declare_dram_parameter("wq2", (128, 3072), BF16, isOutput=False)
    bq_d = nc.declare_dram_parameter("bq", (128, 1), F32, isOutput=False)
    # misc = [wout (1024) | tri|tri (256)]
    misc_d = nc.declare_dram_parameter("misc", (128, 1280), BF16, isOutput=False)
    outp_d = nc.declare_dram_parameter("outp", (1024, T), F16, isOutput=True)

    with TileContext(nc) as tc:
        sb_cm = tc.tile_pool(name="sb", bufs=1)
        sb = sb_cm.__enter__()

        # ---- persistent tiles ----
        wq_cat = sb.tile((128, 3072), BF16, tag="wq")
        xt_cat = sb.tile((128, 32768), BF16, tag="xt")
        bq_t = sb.tile((128, 1), F32, tag="bq")
        misc_t = sb.tile((128, 1280), BF16, tag="misc")
        wout_t = misc_t[:, 0:1024]
        tri2_t = misc_t[:, 1024:1280]
        wrm_t = sb.tile((128, 512), BF16, tag="wrm")
        ones1_t = sb.tile((1, 64), BF16, tag="ones1")

        q_T = sb.tile((128, T), BF16, tag="q_T")
        k_T = sb.tile((128, T), BF16, tag="k_T")
        v_T = sb.tile((128, T), BF16, tag="v_T")
        vext = [
            sb.tile((128, 2080), BF16, tag=f"vext{b}", name=f"vext{b}")
            for b in range(2)
        ]
        # double-buffered per r: block i+1's exp must not wait for block i's
        # AV to release the same diagonal tile
        es_diag = [
            [
                sb.tile((128, 1024), BF16, tag=f"esd{p}{r}", name=f"esd{p}{r}")
                for r in range(4)
            ]
            for p in range(2)
        ]
        av_sb = [
            sb.tile((65, 1024), F32, tag=f"avsb{t}", name=f"avsb{t}") for t in range(8)
        ]
        # per-block softmax denominator reciprocals, computed on ACT as
        # exp(-ln(den)) straight out of the av PSUM row (the ln/exp/copy
        # functions share one activation table: no table reloads)
        recb_t = [
            sb.tile((1, 1024), BF16, tag=f"recbt{t}", name=f"recbt{t}")
            for t in range(8)
        ]

        # ---- input DMAs ----
        # all DMAs ride the two hardware DGE queues (sync + scalar engines);
        # gpsimd DMA is software DGE and costs ~800ns of Pool time per call,
        # which would stall the causal-mask muls that live on Pool. The HWDGE
        # front-end is a single shared device at ~630ns per DMA, so inputs
        # are host-packed down to 11 transfers.
        nc.sync.dma_start(
            bass.AP(
                xt_cat.tensor,
                xt_cat.offset,
                [[xt_cat.ap[0][0], 128], [4096, 8], [1, 512]],
            ),
            xt_d[:, 0:4096],
        )
        nc.scalar.dma_start(wq_cat[:], wq_d[:, :])
        nc.scalar.dma_start(bq_t[:], bq_d[:, :])
        nc.scalar.dma_start(misc_t[:], misc_d[:, :])
        for n in range(1, 8):
            nc.sync.dma_start(
                bass.AP(
                    xt_cat.tensor,
                    xt_cat.offset + 512 * n,
                    [[xt_cat.ap[0][0], 128], [4096, 8], [1, 512]],
                ),
                xt_d[:, 4096 * n : 4096 * n + 4096],
            )

        # ones columns of vext feed the softmax denominator row of av
        nc.vector.memset(wrm_t[:], 0.0)
        nc.vector.memset(ones1_t[:], 1.0)
        nc.vector.memset(vext[0][:], 1.0)
        nc.vector.memset(vext[1][:], 1.0)
        # fully-masked key columns of the diagonal es tiles are zeroed once;
        # exp writes only the unmasked band afterwards
        for p in range(2):
            for r in range(1, 4):
                nc.gpsimd.memset(es_diag[p][r][:, 0 : 128 * r], 0.0)
                nc.gpsimd.memset(es_diag[p][r][:, 512 : 512 + 128 * r], 0.0)

        # PSUM pools form a stack; psA (on top) is released mid-kernel to
        # make room for the out-projection pool psC.
        psB_cm = tc.tile_pool(name="psB", bufs=1, space="PSUM")
        psB = psB_cm.__enter__()
        psA_cm = tc.tile_pool(name="psA", bufs=1, space="PSUM")
        psA = psA_cm.__enter__()

        # ---- warm-up: ramp the PE p-state while the first x slice lands.
        # Runs on a zeroed SBUF tile so it depends on no DMA.
        for w in range(18):
            ch = psA.tile((128, 512), F32, tag="ch", bufs=2, name="chw")
            nc.tensor.matmul(
                ch[:],
                wrm_t[:, 0:128],
                wrm_t[:],
                start=True,
                stop=True,
                skip_group_check=True,
            )

        # ---- QKV projection: one (slice, m) unit = 8 accumulating matmuls ----
        qkv_dst = [q_T, k_T, v_T]

        def emit_A_unit(n, m):
            ch = psA.tile((128, 512), F32, tag="ch", bufs=2, name="ch")
            for kc in range(8):
                nc.tensor.matmul(
                    ch[:],
                    wq_cat[:, 384 * kc + 128 * m : 384 * kc + 128 * m + 128],
                    xt_cat[:, 4096 * kc + 512 * n : 4096 * kc + 512 * n + 512],
                    start=(kc == 0),
                    stop=(kc == 7),
                    skip_group_check=True,
                )
            # all QKV evacuations on DVE: the ACT queue carries the exp
            # chain and the denominator reciprocals; anything extra there
            # delays the next attention block
            o = qkv_dst[m][:, 512 * n : 512 * n + 512]
            if m == 0:
                nc.vector.tensor_scalar_add(o, ch[:], bq_t[:, 0:1])
            else:
                nc.vector.tensor_copy(o, ch[:])

        def emit_V_dma(n):
            # DMA-transpose each 128-token block into a scratch tile (the
            # xbar path only supports plain 2D outputs), then one strided
            # DVE copy scatters both head-halves into vext's 65-wide slots.
            b, i = divmod(n, 4)
            for u in range(4):
                jj = 4 * i + u
                # scalar queue: the sync queue carries the xt stream and the
                # denominator collects, which are latency-critical
                vtr = sb.tile((128, 128), BF16, tag="vtr", bufs=2, name="vtr")
                nc.scalar.dma_start_transpose(
                    vtr[:], v_T[:, 512 * n + 128 * u : 512 * n + 128 * u + 128]
                )
                ovp = vext[b][:, 65 * jj : 65 * jj + 64]
                o_ap = _chunk2(ovp, 1040, 2, 64)
                i_ap = _chunk2(vtr[:, 0:128], 64, 2, 64)
                # DVE, not Pool: the Pool queue carries the causal-mask muls
                # which sit on the exp->AV critical path
                nc.vector.tensor_copy(o_ap, i_ap)

        # filler queue: units of ready PE work fed into attention bubbles.
        # entries are (slice_id_or_None, emit_fn); A-units carry their slice
        # id so attention block i can force slices <= i out first.
        filler = []

        def take_filler(k):
            for _ in range(k):
                if filler:
                    filler.pop(0)[1]()

        def ensure_slices(n):
            while filler and filler[0][0] is not None and filler[0][0] <= n:
                filler.pop(0)[1]()

        def drain_A_units():
            while filler and filler[0][0] is not None:
                filler.pop(0)[1]()

        def drain_filler():
            while filler:
                filler.pop(0)[1]()

        # ---- attention scores+exp for one 128-key block ----
        def emit_B_scores(b, i, jj):
            sps = psB.tile((128, 1024), F32, tag="sps", bufs=2, name="sps")
            ks = 2048 * b + 128 * jj
            qs = 2048 * b + 512 * i
            for hl in range(2):
                nc.tensor.matmul(
                    sps[:, 512 * hl : 512 * hl + 512],
                    k_T[64 * hl : 64 * hl + 64, ks : ks + 128],
                    q_T[64 * hl : 64 * hl + 64, qs : qs + 512],
                    start=True,
                    stop=True,
                    skip_group_check=True,
                )
            r = jj - 4 * i
            if r < 0:
                es = sb.tile((128, 1024), BF16, tag="es", bufs=4, name="es")
                nc.scalar.activation(es[:], sps[:], AF.Exp, scale=0.125)
            else:
                es = es_diag[(4 * b + i) % 2][r]
                if r == 0:
                    nc.scalar.activation(es[:], sps[:], AF.Exp, scale=0.125)
                else:
                    w = 512 - 128 * r
                    o_ap = _chunk2(es[:, 128 * r : 128 * r + w], 512, 2, w)
                    i_ap = _chunk2(sps[:, 128 * r : 128 * r + w], 512, 2, w)
                    nc.scalar.activation(o_ap, i_ap, AF.Exp, scale=0.125)
                # causal triangle on the diagonal 128 columns, both heads.
                # SBUF-only op -> Pool engine, keeping DVE free for PSUM evacs
                e_ap = _chunk2(es[:, 128 * r : 128 * r + 128], 512, 2, 128)
                t_ap = _chunk2(tri2_t[:, 0:128], 128, 2, 128)
                nc.gpsimd.tensor_mul(e_ap, e_ap, t_ap)
            return es

        def emit_B_av(b, av, jj, es, last):
            for hl in range(2):
                c = 65 * (16 * hl + jj)
                nc.tensor.matmul(
                    av[hl][:],
                    vext[b][:, c : c + 65],
                    es[:, 512 * hl : 512 * hl + 512],
                    start=(jj == 0),
                    stop=last,
                    skip_group_check=True,
                )

        pend_recip = [None]

        def emit_recip(t):
            # denominator reciprocal on ACT: 1/den = exp(-ln(den)), read
            # from the evacuated av_sb row (stable across the av PSUM-bank
            # reuse). A DVE reciprocal costs 3.3us on hw and a DMA
            # denominator-collect ~2us more of chain latency.
            lt = sb.tile((1, 1024), F32, tag="lt", bufs=2, name="lt")
            nc.scalar.activation(lt[:], av_sb[t][64:65, 0:1024], AF.Ln)
            nc.scalar.activation(recb_t[t][:], lt[:], AF.Exp, scale=-1.0)

        def pump_B_step(b, i, fill_per_jj=1, fill_floor=0):
            t = 4 * b + i
            nj = 4 * i + 4
            av = [
                psB.tile((65, 512), F32, tag=f"av{hl}", bufs=1, name=f"av{hl}")
                for hl in range(2)
            ]
            pend = None
            for jj in range(nj):
                es = emit_B_scores(b, i, jj)
                if pend is not None:
                    emit_B_av(b, av, pend[0], pend[1], last=False)
                if jj == 2 and pend_recip[0] is not None:
                    # previous block's reciprocal rides the ACT queue here,
                    # behind this block's first exps, so it never delays the
                    # first AV matmuls of this block
                    pend_recip[0]()
                    pend_recip[0] = None
                if fill_per_jj and len(filler) > fill_floor:
                    k = fill_per_jj + (1 if len(filler) > 24 else 0)
                    take_filler(k)
                pend = (jj, es)
            emit_B_av(b, av, pend[0], pend[1], last=True)
            if pend_recip[0] is not None:
                pend_recip[0]()
            # evacuate the attention values (DVE; Pool has no PSUM port)
            nc.vector.tensor_copy(av_sb[t][:, 0:512], av[0][:])
            nc.vector.tensor_copy(av_sb[t][:, 512:1024], av[1][:])
            pend_recip[0] = lambda t=t: emit_recip(t)

        # ---- out-projection units for one query block t ----
        OSB_BUFS = 4

        def emit_C_bcast(t, pool, tag, bufs):
            # two rank-1 matmuls broadcast each head-half's reciprocal row
            # across its 64 partitions (PE quadrant col offset selects rows)
            bcp = pool.tile((128, 512), F32, tag=tag, bufs=bufs, name="bcp")
            for hl in range(2):
                nc.tensor.matmul(
                    bcp[64 * hl : 64 * hl + 64, :],
                    ones1_t[:],
                    recb_t[t][0:1, 512 * hl : 512 * hl + 512],
                    start=True,
                    stop=True,
                    skip_group_check=True,
                    tile_position=(0, 64 * hl),
                )
            attnT = sb.tile((128, 512), BF16, tag="attnT", bufs=2, name="attnT")
            for hl in range(2):
                nc.vector.tensor_mul(
                    attnT[64 * hl : 64 * hl + 64, :],
                    av_sb[t][0:64, 512 * hl : 512 * hl + 512],
                    bcp[64 * hl : 64 * hl + 64, :],
                )
            return attnT

        osb_cur = [None]

        def emit_C_mo(t, mo, attnT, pool, tag, bufs, evac_engs):
            op = pool.tile((128, 512), F32, tag=tag, bufs=bufs, name="op")
            nc.tensor.matmul(
                op[:],
                wout_t[:, 128 * mo : 128 * mo + 128],
                attnT[:],
                start=True,
                stop=True,
                skip_group_check=True,
            )
            # evacuate mo-pairs into one (128,1024) staging tile; a single
            # DMA then scatters 256 output rows (keeps the HWDGE count low)
            if mo % 2 == 0:
                osb_cur[0] = sb.tile(
                    (128, 1024), F16, tag="osb", bufs=OSB_BUFS, name="osb"
                )
            osb = osb_cur[0]
            half = mo % 2
            eng = evac_engs[mo % len(evac_engs)]
            dst = osb[:, 512 * half : 512 * half + 512]
            if eng == "v":
                nc.vector.tensor_copy(dst, op[:])
            else:
                nc.scalar.activation(dst, op[:], AF.Copy)
            if half == 1:
                src = _chunk2(osb[:, 0:1024], 512, 2, 512)
                base = outp_d[0:128, 0:512]
                doff = base.offset + 128 * (mo - 1) * T + 512 * t
                dap = bass.AP(
                    base.tensor, doff, [[T, 128], [128 * T, 2], [1, 512]]
                )
                deng = nc.sync if mo % 4 == 1 else nc.scalar
                deng.dma_start(dap, src)

        # =====================  schedule  =====================
        # QKV slice 0 up front; slices 1..7 and each finished block's
        # out-projection are queued as PE filler consumed inside the
        # attention pumps. The out-projection of block t shares the psA
        # "ch" PSUM ring with the QKV accumulation units.
        c_attnT = {}

        def emit_C_unit(u):
            if u[0] == "b":
                # the broadcast reads recb_t; flush a still-pending
                # reciprocal so the read binds to the right write
                if pend_recip[0] is not None:
                    pend_recip[0]()
                    pend_recip[0] = None
                c_attnT[u[1]] = emit_C_bcast(u[1], psA, "ch", 2)
            else:
                _, t, mo = u
                emit_C_mo(t, mo, c_attnT[t], psA, "ch", 2, ["v"])

        for m in range(3):
            emit_A_unit(0, m)
        emit_V_dma(0)
        for n in range(1, 8):
            for m in range(3):
                filler.append((n, lambda n=n, m=m: emit_A_unit(n, m)))
            filler.append((n, lambda n=n: emit_V_dma(n)))

        for b in range(2):
            for i in range(4):
                t = 4 * b + i
                ensure_slices(4 * b + i)
                # hold ~8 units back at the end to cover the final recip +
                # pool-transition window before block 7's out-projection
                pump_B_step(b, i, fill_per_jj=1, fill_floor=10)
                if t < 7:
                    filler.append((None, lambda t=t: emit_C_unit(("b", t))))
                    for mo in range(8):
                        filler.append(
                            (None, lambda t=t, mo=mo: emit_C_unit(("m", t, mo)))
                        )
        if pend_recip[0] is not None:
            pend_recip[0]()
            pend_recip[0] = None
        drain_filler()

        psA_cm.__exit__(None, None, None)
        psB_cm.__exit__(None, None, None)
        psC2_cm = tc.tile_pool(name="psC2", bufs=1, space="PSUM")
        psC2 = psC2_cm.__enter__()

        # final block's out-projection with deep rings (nothing left to hide
        # behind, so give the evacuation maximum pipeline depth)
        attnT7 = emit_C_bcast(7, psC2, "bcp2", 2)
        for mo in range(8):
            emit_C_mo(7, mo, attnT7, psC2, "op2", 4, ["s", "v"])

        psC2_cm.__exit__(None, None, None)
        sb_cm.__exit__(None, None, None)

    _split_excess_waits(nc)
    return nc


def _prepare_inmaps(inputs):
    x = np.asarray(inputs["x"], np.float32)
    Wqkv = np.asarray(inputs["W_qkv"], np.float32)
    bqkv = np.asarray(inputs["b_qkv"], np.float32)
    Wout = np.asarray(inputs["W_out"], np.float32)

    # xt2[p, 4096*n + 512*kc + j] = x^T[128*kc + p, 512*n + j]
    xt = x.reshape(T, 1024).T.reshape(8, 128, 8, 512)  # (kc, p, n, j)
    xt2 = np.ascontiguousarray(xt.transpose(1, 2, 0, 3).reshape(128, 32768)).astype(
        BF
    )

    kk = np.arange(128)[:, None]
    qq = np.arange(128)[None, :]
    tri = (qq >= kk).astype(np.float32)
    tri2 = np.concatenate([tri, tri], axis=1)

    in_maps = []
    for c in range(8):
        s = 128 * c
        wq = np.concatenate(
            [
                Wqkv[:, s : s + 128],
                Wqkv[:, 1024 + s : 1024 + s + 128],
                Wqkv[:, 2048 + s : 2048 + s + 128],
            ],
            axis=1,
        )  # (1024, 384)
        # wq2[p, 384*kc + col] = wq[128*kc + p, col]
        wq2 = np.ascontiguousarray(
            wq.reshape(8, 128, 384).transpose(1, 0, 2).reshape(128, 3072)
        ).astype(BF)
        misc = np.ascontiguousarray(
            np.concatenate([Wout[s : s + 128, :], tri2], axis=1)
        ).astype(BF)
        in_maps.append(
            {
                "xt2": xt2,
                "wq2": wq2,
                "bq": np.ascontiguousarray(
                    bqkv[s : s + 128].reshape(128, 1)
                ).astype(np.float32),
                "misc": misc,
            }
        )
    return in_maps


def _postprocess(results, inputs):
    bqkv = np.asarray(inputs["b_qkv"], np.float32)
    Wout = np.asarray(inputs["W_out"], np.float32)
    bout = np.asarray(inputs["b_out"], np.float32)
    total = np.zeros((1024, T), np.float32)
    for c in range(8):
        total += np.asarray(results[c]["outp"]).astype(np.float32)
    beff = (
        bout.astype(np.float64)
        + bqkv[2048:].astype(np.float64) @ Wout.astype(np.float64)
    ).astype(np.float32)
    out = total.T.reshape(2, 2048, 1024) + beff
    return out.astype(np.float32)


def _numpy_partials(inputs, cores):
    """Per-core expected partial outp (float32, [1024, T]) replicating the
    device math in numpy (for subset sim checks)."""
    x = np.asarray(inputs["x"], np.float64)
    Wqkv = np.asarray(inputs["W_qkv"], np.float64)
    bqkv = np.asarray(inputs["b_qkv"], np.float64)
    Wout = np.asarray(inputs["W_out"], np.float64)
    xt = x.reshape(T, 1024)
    out = {}
    for c in cores:
        s = 128 * c
        q = xt @ Wqkv[:, s : s + 128] + bqkv[s : s + 128]
        k = xt @ Wqkv[:, 1024 + s : 1024 + s + 128]
        v = xt @ Wqkv[:, 2048 + s : 2048 + s + 128]
        part = np.zeros((T, 1024))
        for b in range(2):
            sl = slice(2048 * b, 2048 * b + 2048)
            for h in range(2):
                hs = slice(64 * h, 64 * h + 64)
                sc = q[sl, hs] @ k[sl, hs].T / 8.0
                mask = np.tril(np.ones((2048, 2048), dtype=bool))
                sc = np.where(mask, sc, -np.inf)
                w = np.exp(sc - sc.max(axis=1, keepdims=True))
                w /= w.sum(axis=1, keepdims=True)
                part[sl] += (w @ v[sl, hs]) @ Wout[s + 64 * h : s + 64 * h + 64, :]
        out[c] = part.T.astype(np.float32)
    return out


def kernel(**inputs):
    global LAST_EXEC_NS, LAST_MEAN_NS
    in_maps = _prepare_inmaps(inputs)
    nc = _build()
    res = run_bass_kernel_spmd(nc, in_maps, list(range(8)), trace=TRACE)
    LAST_EXEC_NS = res.exec_time_ns
    LAST_MEAN_NS = res.mean_exec_time_ns
    return _postprocess(res.results, inputs)
